# revision 2
# baseline (speedup 1.0000x reference)
"""Bahdanau-style attention for Trainium2, data-parallel over batch on 8 NeuronCores.

Reference computes::

    energy[b,t,k] = hidden[b]@Wh^T + enc[b,t]@We^T + b_attn      (Wh|We = W_attn split)
    scores[b,t]   = energy[b,t,:]@W_v[0] + b_v
    ctx[b]        = softmax_t(scores[b]) @ enc[b]

scores is affine in energy, so fold W_v into the weights:
``scores[b,t] = hidden[b].u_h + enc[b,t].u_e + c`` with ``u_e = W_v[0] @ We``.
The ``hidden.u_h + c`` part is constant across t and cancels in the softmax, so

    ctx[b] = softmax_t(enc[b] @ u_e) @ enc[b]

which needs exactly one HBM pass over enc (256 MB total, 32 MB/core) — the whole
kernel is DMA-bound.  Per 128-timestep tile (SBUF layout [t=128 part, h=1024 free]):

  * scores: one fused DVE ``tensor_tensor_reduce`` (mult + add-reduce over h)
  * exp:    ScalarE activation
  * ctx:    PE matmul ``e_col.T @ enc_tile`` accumulating into PSUM [1, 1024];
            Z (softmax denominator) via a ones-vector matmul, final scale by 1/Z.

No max-subtraction: scores are enc.u_e with |score| < ~3 for any plausible input
scale here (weights are tiny nn.Linear-init bounds), far from exp overflow.
"""

import numpy as np

import concourse.bacc as bacc
import concourse.tile as tile
from concourse import mybir
from concourse.bass_utils import run_bass_kernel_spmd

H = 1024
B = 32
T = 2048
NCORES = 8
BL = B // NCORES        # batches per core
P = 128                 # SBUF partitions
NT = T // P             # 16 t-tiles per batch
CT = 4                  # t-tiles per DMA chunk (2 MiB)
NCHUNK = NT // CT
F32 = mybir.dt.float32

_NC = None


def _build():
    nc = bacc.Bacc("TRN2", target_bir_lowering=False, debug=False)
    enc = nc.dram_tensor("enc", [BL, T, H], F32, kind="ExternalInput")
    u = nc.dram_tensor("u", [P, H], F32, kind="ExternalInput")
    out = nc.dram_tensor("out", [BL, H], F32, kind="ExternalOutput")

    with tile.TileContext(nc) as tc:
        with (
            tc.tile_pool(name="chunks", bufs=6) as chunks,
            tc.tile_pool(name="singles", bufs=1) as singles,
            tc.tile_pool(name="small", bufs=2) as small,
            tc.tile_pool(name="psum_ctx", bufs=2, space="PSUM") as pc_pool,
            tc.tile_pool(name="psum_z", bufs=2, space="PSUM") as pz_pool,
        ):
            u_sb = singles.tile([P, H], F32)
            nc.sync.dma_start(out=u_sb[:], in_=u[:])
            ones_sb = singles.tile([P, 1], F32)
            nc.vector.memset(ones_sb[:], 1.0)

            for b in range(BL):
                s_tile = small.tile([P, NT], F32, tag="scores")
                e_tile = small.tile([P, NT], F32, tag="exps")
                dummy = small.tile([P, 1], F32, tag="ttr_dummy")
                psum_ctx = pc_pool.tile([1, H], F32, tag="ctx")
                psum_z = pz_pool.tile([1, CT], F32, tag="z")

                enc_b = enc[b].rearrange("(n p) h -> p n h", p=P)  # [P, NT, H]

                for c in range(NCHUNK):
                    chunk = chunks.tile([P, CT, H], F32, tag="chunk")
                    nc.sync.dma_start(
                        out=chunk[:],
                        in_=enc_b[:, c * CT : (c + 1) * CT, :],
                    )
                    for j in range(CT):
                        i = c * CT + j
                        nc.vector.affine_mul_reduce(
                            out=dummy.broadcast_to((P, H)),
                            accum_out=s_tile[:, i : i + 1],
                            in0=chunk[:, j, :],
                            in1=u_sb[:],
                            scale=1.0,
                            bias=0.0,
                        )
                    nc.scalar.activation(
                        out=e_tile[:, c * CT : (c + 1) * CT],
                        in_=s_tile[:, c * CT : (c + 1) * CT],
                        func=mybir.ActivationFunctionType.Exp,
                    )
                    for j in range(CT):
                        i = c * CT + j
                        for ns in range(H // 512):
                            nc.tensor.matmul(
                                psum_ctx[:, ns * 512 : (ns + 1) * 512],
                                lhsT=e_tile[:, i : i + 1],
                                rhs=chunk[:, j, ns * 512 : (ns + 1) * 512],
                                start=(i == 0),
                                stop=(i == NT - 1),
                            )
                    nc.tensor.matmul(
                        psum_z[:],
                        lhsT=ones_sb[:],
                        rhs=e_tile[:, c * CT : (c + 1) * CT],
                        start=(c == 0),
                        stop=(c == NCHUNK - 1),
                    )

                z_sum = small.tile([1, 1], F32, tag="zsum")
                nc.vector.reduce_sum(
                    out=z_sum[:], in_=psum_z[:], axis=mybir.AxisListType.X
                )
                rz = small.tile([1, 1], F32, tag="rz")
                nc.vector.reciprocal(rz[:], z_sum[:])
                ctx_sb = small.tile([1, H], F32, tag="ctx_sb")
                nc.scalar.activation(
                    out=ctx_sb[:],
                    in_=psum_ctx[:],
                    func=mybir.ActivationFunctionType.Copy,
                    scale=rz[:],
                )
                nc.sync.dma_start(out=out[b : b + 1, :], in_=ctx_sb[:])

    nc.compile()
    return nc


def _get_nc():
    global _NC
    if _NC is None:
        _NC = _build()
    return _NC


def _make_in_maps(encoder_outputs, W_attn, W_v):
    u_e = (W_v[0].astype(np.float64) @ W_attn[:, H:].astype(np.float64)).astype(
        np.float32
    )
    u_host = np.ascontiguousarray(np.tile(u_e[None, :], (P, 1)))
    return [
        {
            "enc": np.ascontiguousarray(encoder_outputs[c * BL : (c + 1) * BL]),
            "u": u_host,
        }
        for c in range(NCORES)
    ]


def kernel(encoder_outputs, hidden, W_attn, b_attn, W_v, b_v):
    encoder_outputs = np.asarray(encoder_outputs, dtype=np.float32)
    W_attn = np.asarray(W_attn, dtype=np.float32)
    W_v = np.asarray(W_v, dtype=np.float32)
    nc = _get_nc()
    in_maps = _make_in_maps(encoder_outputs, W_attn, W_v)
    res = run_bass_kernel_spmd(nc, in_maps, core_ids=list(range(NCORES)))
    return np.concatenate([r["out"] for r in res.results], axis=0)


# revision 10
# speedup vs baseline: 29.6594x; 29.6594x over previous
"""Bahdanau-style attention for Trainium2, data-parallel over batch on 8 NeuronCores.

Reference computes::

    energy[b,t,k] = hidden[b]@Wh^T + enc[b,t]@We^T + b_attn      (Wh|We = W_attn split)
    scores[b,t]   = energy[b,t,:]@W_v[0] + b_v
    ctx[b]        = softmax_t(scores[b]) @ enc[b]

scores is affine in energy, so fold W_v into the weights:
``scores[b,t] = hidden[b].u_h + enc[b,t].u_e + c`` with ``u_e = W_v[0] @ We``.
The ``hidden.u_h + c`` part is constant across t and cancels in the softmax, so

    ctx[b] = softmax_t(enc[b] @ u_e) @ enc[b]

which needs exactly one HBM pass over enc (256 MB total, 32 MB/core).  Per
128-timestep tile (SBUF layout [t=128 part, h=1024 free]):

  * scores: one fused DVE ``affine_mul_reduce`` (mult + add-reduce over h)
  * exp:    ScalarE activation (written as float32r for the PE)
  * ctx:    PE matmul ``e_col.T @ enc_tile`` accumulating into PSUM [1, 1024];
            Z (softmax denominator) via a ones-vector matmul, final scale by 1/Z.

The context matmuls run in float32r (single-pass fp32, 1 cycle/column) instead
of exact fp32 (2-pass, 4 cycles/column) — exact fp32 made the PE stream the
kernel bottleneck at ~105 us/core (HW-measured; walrus requires fp32r-matmul
operands to be produced as fp32r, hence the fp32r tile dtypes + bitcasts).
PSUM still accumulates in fp32; HW rel err vs the fp32 reference is 7e-5.

enc is streamed in 4 MiB chunks (8 t-tiles), triple-buffered: one dma_start
fans out over all 16 SDMA engines, and 4 MiB amortizes per-descriptor overhead
(2 MiB chunks measured ~13 us/iter slower).  Steady-state HW time is ~91 us
per core = ~368 GB/s — at the ~358 GB/s HBM-per-NeuronCore roofline.

No max-subtraction: scores are enc.u_e with |score| < ~3 for any plausible input
scale here (weights are tiny nn.Linear-init bounds), far from exp overflow.
"""

import numpy as np

import concourse.bacc as bacc
import concourse.tile as tile
from concourse import mybir
from concourse.bass_utils import run_bass_kernel_spmd

H = 1024
B = 32
T = 2048
NCORES = 8
BL = B // NCORES        # batches per core
P = 128                 # SBUF partitions
NT = T // P             # 16 t-tiles per batch
CT = 8                  # t-tiles per DMA chunk (4 MiB)
NCHUNK = NT // CT
F32 = mybir.dt.float32
FR = mybir.dt.float32r

_NC = None


def _build(repeats=1):
    nc = bacc.Bacc("TRN2", target_bir_lowering=False, debug=False)
    enc = nc.dram_tensor("enc", [BL, T, H], F32, kind="ExternalInput")
    u = nc.dram_tensor("u", [P, H], F32, kind="ExternalInput")
    ones_in = nc.dram_tensor("ones", [P, 1], F32, kind="ExternalInput")
    out = nc.dram_tensor("out", [BL, H], F32, kind="ExternalOutput")

    with tile.TileContext(nc) as tc:
        with (
            tc.tile_pool(name="chunks", bufs=3) as chunks,
            tc.tile_pool(name="singles", bufs=1) as singles,
            tc.tile_pool(name="small", bufs=2) as small,
            tc.tile_pool(name="psum_ctx", bufs=2, space="PSUM") as pc_pool,
            tc.tile_pool(name="psum_z", bufs=2, space="PSUM") as pz_pool,
        ):
            u_sb = singles.tile([P, H], F32)
            nc.sync.dma_start(out=u_sb[:], in_=u[:])
            ones_fr = singles.tile([P, 1], FR)
            nc.sync.dma_start(out=ones_fr[:], in_=ones_in[:].bitcast(FR))

            for b in [b for _ in range(repeats) for b in range(BL)]:
                s_tile = small.tile([P, NT], F32, tag="scores")
                e_tile = small.tile([P, NT], FR, tag="exps")
                dummy = small.tile([P, 1], F32, tag="ttr_dummy")
                psum_ctx = pc_pool.tile([1, H], F32, tag="ctx")
                psum_z = pz_pool.tile([1, CT], F32, tag="z")

                enc_b = enc[b].rearrange("(n p) h -> p n h", p=P)  # [P, NT, H]

                for c in range(NCHUNK):
                    chunk = chunks.tile([P, CT, H], FR, tag="chunk")
                    nc.sync.dma_start(
                        out=chunk[:],
                        in_=enc_b[:, c * CT : (c + 1) * CT, :].bitcast(FR),
                    )
                    for j in range(CT):
                        i = c * CT + j
                        nc.vector.affine_mul_reduce(
                            out=dummy.broadcast_to((P, H)),
                            accum_out=s_tile[:, i : i + 1],
                            in0=chunk[:, j, :].bitcast(F32),
                            in1=u_sb[:],
                            scale=1.0,
                            bias=0.0,
                        )
                    nc.scalar.activation(
                        out=e_tile[:, c * CT : (c + 1) * CT],
                        in_=s_tile[:, c * CT : (c + 1) * CT],
                        func=mybir.ActivationFunctionType.Exp,
                    )
                    for j in range(CT):
                        i = c * CT + j
                        for ns in range(H // 512):
                            nc.tensor.matmul(
                                psum_ctx[:, ns * 512 : (ns + 1) * 512],
                                lhsT=e_tile[:, i : i + 1],
                                rhs=chunk[:, j, ns * 512 : (ns + 1) * 512],
                                start=(i == 0),
                                stop=(i == NT - 1),
                            )
                    nc.tensor.matmul(
                        psum_z[:],
                        lhsT=ones_fr[:],
                        rhs=e_tile[:, c * CT : (c + 1) * CT],
                        start=(c == 0),
                        stop=(c == NCHUNK - 1),
                    )

                z_sum = small.tile([1, 1], F32, tag="zsum")
                nc.vector.reduce_sum(
                    out=z_sum[:], in_=psum_z[:], axis=mybir.AxisListType.X
                )
                rz = small.tile([1, 1], F32, tag="rz")
                nc.vector.reciprocal(rz[:], z_sum[:])
                ctx_sb = small.tile([1, H], F32, tag="ctx_sb")
                nc.scalar.activation(
                    out=ctx_sb[:],
                    in_=psum_ctx[:],
                    func=mybir.ActivationFunctionType.Copy,
                    scale=rz[:],
                )
                nc.sync.dma_start(out=out[b : b + 1, :], in_=ctx_sb[:])

    nc.compile()
    return nc


def _get_nc():
    global _NC
    if _NC is None:
        _NC = _build()
    return _NC


def _make_in_maps(encoder_outputs, W_attn, W_v):
    u_e = (W_v[0].astype(np.float64) @ W_attn[:, H:].astype(np.float64)).astype(
        np.float32
    )
    u_host = np.ascontiguousarray(np.tile(u_e[None, :], (P, 1)))
    ones_host = np.ones((P, 1), dtype=np.float32)
    return [
        {
            "enc": np.ascontiguousarray(encoder_outputs[c * BL : (c + 1) * BL]),
            "u": u_host,
            "ones": ones_host,
        }
        for c in range(NCORES)
    ]


def kernel(encoder_outputs, hidden, W_attn, b_attn, W_v, b_v):
    encoder_outputs = np.asarray(encoder_outputs, dtype=np.float32)
    W_attn = np.asarray(W_attn, dtype=np.float32)
    W_v = np.asarray(W_v, dtype=np.float32)
    nc = _get_nc()
    in_maps = _make_in_maps(encoder_outputs, W_attn, W_v)
    res = run_bass_kernel_spmd(nc, in_maps, core_ids=list(range(NCORES)))
    return np.concatenate([r["out"] for r in res.results], axis=0)
